# revision 1
# baseline (speedup 1.0000x reference)
"""DeepSeek-V2 MLA decoder layer (prefill, T=2048) on 8 Trainium2 NeuronCores.

Strategy (validated against reference by a numpy golden model):
  Stage 1 (token-parallel, 256 tok/core): qkv_a proj + RMSNorms + k_pe rope,
     outputs feature-major fp16; kv/pe columns are computed first and
     all-gathered early (AG2) so stage-2 k/v builds overlap the q AllGather.
  Stage 2 (head-parallel, 4 heads/core): q/kv b-projections, q rope,
     causal attention computed pre-transposed (S^T = k^T q per tile; row sums
     via a ones-vector matmul accumulated in PSUM; no-max softmax with 2^-7
     bias), partial o_proj with the local heads' w_o rows.
  Host: sum the 8 partial outputs.

All matmuls fp16 inputs / fp32 PSUM accumulation. LN weights are folded into
the b-projection weights on the host; rope pairs are de-interleaved (E/O) by
host-side weight column permutation so rope becomes contiguous-block math.
"""
import numpy as np

import concourse.bass as bass
import concourse.mybir as mybir
import concourse.tile as tile
from concourse import bacc
from concourse.bass_utils import run_bass_kernel_spmd
from concourse.masks import make_identity

F16 = mybir.dt.float16
F32 = mybir.dt.float32
AX = mybir.AxisListType
AF = mybir.ActivationFunctionType

NCORES = 8
T, HID, H = 2048, 5120, 32
DN, DR, DV, QL, KL = 128, 64, 128, 1536, 512
EPS = 1e-6
THETA = 10000.0
HPC = H // NCORES            # 4 heads per core
TPC = T // NCORES            # 256 tokens per core
CW = QL + KL + DR            # 2112
KVW = KL + DR                # 576 (early AG payload rows)
SM_SCALE = float((DN + DR) ** -0.5)
EXP_BIAS = float(-7.0 * np.log(2.0))   # exp(s*scale - 7 ln2): fp16-safe range
NEG = -1e9
QTILES = T // 128            # 16

_PROGRAM_CACHE = {}


def build_program():
    if "nc" in _PROGRAM_CACHE:
        return _PROGRAM_CACHE["nc"]
    nc = bacc.Bacc("TRN2", target_bir_lowering=False, debug=False,
                   num_devices=NCORES)

    hT_d = nc.dram_tensor("hT", [HID, TPC], F16, kind="ExternalInput").ap()
    wa_d = nc.dram_tensor("wa", [HID, CW], F16, kind="ExternalInput").ap()
    wqb_d = nc.dram_tensor("wqb", [QL, HPC * 192], F16, kind="ExternalInput").ap()
    wkvb_d = nc.dram_tensor("wkvb", [KL, HPC * 256], F16, kind="ExternalInput").ap()
    wo_d = nc.dram_tensor("wo", [HPC * DV, HID], F16, kind="ExternalInput").ap()
    ctok_d = nc.dram_tensor("ctok", [TPC, 32], F16, kind="ExternalInput").ap()
    stok_d = nc.dram_tensor("stok", [TPC, 32], F16, kind="ExternalInput").ap()
    cosF4_d = nc.dram_tensor("cosF4", [128, T], F32, kind="ExternalInput").ap()
    sinF4_d = nc.dram_tensor("sinF4", [128, T], F32, kind="ExternalInput").ap()
    triT_d = nc.dram_tensor("triT", [128, 128], F32, kind="ExternalInput").ap()
    out_d = nc.dram_tensor("out", [T, HID], F16, kind="ExternalOutput").ap()

    with tile.TileContext(nc) as tc:
        with (
            tc.tile_pool(name="const", bufs=1) as cst,
            tc.tile_pool(name="dram", bufs=1, space="DRAM") as dram,
            tc.tile_pool(name="dram2", bufs=2, space="DRAM") as dram2,
            tc.tile_pool(name="attn_out", bufs=1) as aout,
            tc.tile_pool(name="qkvres", bufs=1) as res,
        ):
            ident16 = cst.tile([128, 128], F16, tag="id16")
            make_identity(nc, ident16[:])
            ones16 = cst.tile([128, 1], F16, tag="ones16")
            nc.vector.memset(ones16[:], 1.0)
            triT_sb = cst.tile([128, 128], F32, tag="triT")
            nc.sync.dma_start(triT_sb[:], triT_d[:])
            ctok_sb = cst.tile([128, 2, 32], F16, tag="ctok")
            nc.sync.dma_start(ctok_sb[:], ctok_d.rearrange("(a p) f -> p a f", p=128))
            stok_sb = cst.tile([128, 2, 32], F16, tag="stok")
            nc.sync.dma_start(stok_sb[:], stok_d.rearrange("(a p) f -> p a f", p=128))
            cosF4_sb = cst.tile([128, T], F32, tag="cosF4")
            nc.sync.dma_start(cosF4_sb[:], cosF4_d[:])
            sinF4_sb = cst.tile([128, T], F32, tag="sinF4")
            nc.sync.dma_start(sinF4_sb[:], sinF4_d[:])
            eps_sb = cst.tile([128, 1], F32, tag="eps")
            nc.vector.memset(eps_sb[:], EPS)
            ebias_sb = cst.tile([128, 1], F32, tag="ebias")
            nc.vector.memset(ebias_sb[:], EXP_BIAS)

            ag1_in = dram.tile([QL, TPC], F16, tag="ag1in")
            ag1_out = dram.tile([NCORES * QL, TPC], F16, addr_space="Shared",
                                tag="ag1out")
            ag2_in = dram.tile([KVW, TPC], F16, tag="ag2in")
            ag2_out = dram.tile([NCORES * KVW, TPC], F16, addr_space="Shared",
                                tag="ag2out")

            # persistent stage-2 tensors
            attnT = [aout.tile([128, T], F16, tag=f"attnT{h}", name=f"attnT{h}")
                     for h in range(HPC)]
            qTn = [res.tile([128, T], F16, tag=f"qTn{h}", name=f"qTn{h}")
                   for h in range(HPC)]
            qTpe = [res.tile([64, T], F16, tag=f"qTpe{h}", name=f"qTpe{h}")
                    for h in range(HPC)]
            kT = [res.tile([128, T], F16, tag=f"kT{h}", name=f"kT{h}")
                  for h in range(HPC)]
            kpeT = res.tile([64, T], F16, tag="kpeT")
            v_sb = res.tile([128, QTILES, HPC * DV], F16, tag="v_sb")
            wo_sb = res.tile([128, HPC, HID], F16, tag="wo_sb")

            # ---------------- Stage 1: token-parallel qkv_a + norms + kpe rope
            with (
                tc.tile_pool(name="ph1", bufs=1) as ph1,
                tc.tile_pool(name="ph1w", bufs=4) as ph1w,
                tc.tile_pool(name="ph1s", bufs=4) as ph1s,
                tc.tile_pool(name="ph1ps", bufs=2, space="PSUM") as ph1ps,
            ):
                hT_sb = ph1.tile([128, HID // 128, TPC], F16, tag="hT")
                hT_r = hT_d.rearrange("(ko p) t -> p ko t", p=128)
                for kg in range(4):
                    nc.sync.dma_start(hT_sb[:, kg * 10:(kg + 1) * 10, :],
                                      hT_r[:, kg * 10:(kg + 1) * 10, :])
                stage = [ph1.tile([128, CW], F16, tag=f"stage{tt}", name=f"stage{tt}")
                         for tt in range(2)]

                def mm_slices(slices):
                    for n0, w in slices:
                        ps = [ph1ps.tile([128, w], F32, tag=f"s1ps{tt}",
                                         name=f"s1ps{tt}") for tt in range(2)]
                        for kg in range(HID // 512):
                            wa_t = ph1w.tile([128, 4, w], F16, tag="wa_t",
                                             name="wa_t")
                            src = bass.AP(
                                tensor=wa_d.tensor,
                                offset=wa_d.offset + kg * 512 * CW + n0,
                                ap=[[CW, 128], [128 * CW, 4], [1, w]])
                            nc.sync.dma_start(wa_t[:], src)
                            for j in range(4):
                                kc = kg * 4 + j
                                for tt in range(2):
                                    nc.tensor.matmul(
                                        ps[tt][:],
                                        hT_sb[:, kc, tt * 128:(tt + 1) * 128],
                                        wa_t[:, j, :], start=(kc == 0),
                                        stop=(kc == HID // 128 - 1))
                        for tt in range(2):
                            nc.scalar.copy(stage[tt][:, n0:n0 + w], ps[tt][:])

                def transpose_to(dst, src, tt, nblk, row0=0):
                    # src [128, 128*nblk] fp16 -> dst dram rows, cols tt*128
                    for b in range(nblk):
                        tp = ph1ps.tile([128, 128], F16, tag="s1tp", name="s1tp")
                        nc.tensor.transpose(tp[:], src[:, b * 128:(b + 1) * 128],
                                            ident16[:])
                        tb = ph1s.tile([128, 128], F16, tag="s1tb", name="s1tb")
                        nc.vector.tensor_copy(tb[:], tp[:])
                        nc.sync.dma_start(
                            dst[row0 + b * 128:row0 + (b + 1) * 128,
                                tt * 128:(tt + 1) * 128],
                            tb[:])

                # --- kv + pe first (feeds the early AllGather)
                mm_slices([(QL, KL), (QL + KL, DR)])
                for tt in range(2):
                    sums = ph1s.tile([128, 4], F32, tag="s1sums")
                    dump = ph1s.tile([128, 512], F16, tag="s1dump")
                    nc.scalar.activation(dump[:], stage[tt][:, QL:QL + KL],
                                         AF.Square, accum_out=sums[:, 3:4])
                    rkv = ph1s.tile([128, 1], F32, tag="rkv")
                    nc.scalar.activation(rkv[:], sums[:, 3:4], AF.Sqrt,
                                         bias=eps_sb[:], scale=1.0 / KL)
                    nc.vector.reciprocal(rkv[:], rkv[:])
                    kva16 = ph1.tile([128, KL], F16, tag=f"kva16_{tt}",
                                     name=f"kva16_{tt}")
                    nc.scalar.activation(kva16[:], stage[tt][:, QL:QL + KL],
                                         AF.Copy, scale=rkv[:])
                    # k_pe rope (host permuted cols to [E32|O32])
                    kpe16 = ph1.tile([128, 64], F16, tag=f"kpe16_{tt}",
                                     name=f"kpe16_{tt}")
                    pe = stage[tt][:, QL + KL:CW]
                    ct, st = ctok_sb[:, tt, :], stok_sb[:, tt, :]
                    t1 = ph1s.tile([128, 32], F32, tag="rt1")
                    t2 = ph1s.tile([128, 32], F32, tag="rt2")
                    nc.vector.tensor_mul(t1[:], pe[:, 0:32], ct)
                    nc.vector.tensor_mul(t2[:], pe[:, 32:64], st)
                    nc.vector.tensor_sub(kpe16[:, 0:32], t1[:], t2[:])
                    t3 = ph1s.tile([128, 32], F32, tag="rt3")
                    t4 = ph1s.tile([128, 32], F32, tag="rt4")
                    nc.vector.tensor_mul(t3[:], pe[:, 32:64], ct)
                    nc.vector.tensor_mul(t4[:], pe[:, 0:32], st)
                    nc.vector.tensor_add(kpe16[:, 32:64], t3[:], t4[:])

                    transpose_to(ag2_in, kva16, tt, 4)
                    tp2 = ph1ps.tile([64, 128], F16, tag="s1tp2")
                    nc.tensor.transpose(tp2[:], kpe16[:], ident16[:])
                    tb2 = ph1s.tile([64, 128], F16, tag="s1tb2")
                    nc.vector.tensor_copy(tb2[:], tp2[:])
                    nc.sync.dma_start(ag2_in[KL:KVW, tt * 128:(tt + 1) * 128],
                                      tb2[:])

                nc.gpsimd.collective_compute(
                    "AllGather", mybir.AluOpType.bypass,
                    ins=[ag2_in.opt()], outs=[ag2_out.opt()],
                    replica_groups=[list(range(NCORES))])

                # --- q part
                mm_slices([(0, 512), (512, 512), (1024, 512)])
                for tt in range(2):
                    sums = ph1s.tile([128, 4], F32, tag="s1sums")
                    dump = ph1s.tile([128, 512], F16, tag="s1dump")
                    for i in range(3):
                        nc.scalar.activation(dump[:],
                                             stage[tt][:, i * 512:(i + 1) * 512],
                                             AF.Square, accum_out=sums[:, i:i + 1])
                    qs = ph1s.tile([128, 1], F32, tag="qs")
                    nc.vector.reduce_sum(qs[:], sums[:, 0:3], axis=AX.X)
                    rq = ph1s.tile([128, 1], F32, tag="rq")
                    nc.scalar.activation(rq[:], qs[:], AF.Sqrt, bias=eps_sb[:],
                                         scale=1.0 / QL)
                    nc.vector.reciprocal(rq[:], rq[:])
                    qa16 = ph1.tile([128, QL], F16, tag=f"qa16_{tt}",
                                    name=f"qa16_{tt}")
                    for i in range(3):
                        nc.scalar.activation(qa16[:, i * 512:(i + 1) * 512],
                                             stage[tt][:, i * 512:(i + 1) * 512],
                                             AF.Copy, scale=rq[:])
                    transpose_to(ag1_in, qa16, tt, 12)

                nc.gpsimd.collective_compute(
                    "AllGather", mybir.AluOpType.bypass,
                    ins=[ag1_in.opt()], outs=[ag1_out.opt()],
                    replica_groups=[list(range(NCORES))])

            def ag1_rows(row0, nrows, r):
                return ag1_out[r * QL + row0: r * QL + row0 + nrows, :]

            def ag2_rows(row0, nrows, r):
                return ag2_out[r * KVW + row0: r * KVW + row0 + nrows, :]

            # ---------------- Stage 2a/2b: projections
            with (
                tc.tile_pool(name="proj", bufs=1) as proj,
                tc.tile_pool(name="projw", bufs=3) as projw,
                tc.tile_pool(name="projs", bufs=1) as projs,
            ):
                # kT (feature-major) and v (token-major) from the early AG
                wkvb_sb = proj.tile([128, KL // 128, HPC * 256], F16, tag="wkvb")
                nc.sync.dma_start(wkvb_sb[:],
                                  wkvb_d.rearrange("(ko p) c -> p ko c", p=128))
                kvaT_sb = [proj.tile([128, T], F16, tag=f"kvaT{cc}",
                                     name=f"kvaT{cc}") for cc in range(KL // 128)]
                for cc in range(KL // 128):
                    src = bass.AP(tensor=ag2_out.tensor,
                                  offset=ag2_out.offset + cc * 128 * TPC,
                                  ap=[[TPC, 128], [KVW * TPC, NCORES], [1, TPC]])
                    nc.sync.dma_start(
                        kvaT_sb[cc][:].rearrange("p (r t) -> p r t", r=NCORES), src)
                src = bass.AP(tensor=ag2_out.tensor,
                              offset=ag2_out.offset + KL * TPC,
                              ap=[[TPC, 64], [KVW * TPC, NCORES], [1, TPC]])
                nc.sync.dma_start(
                    kpeT[:].rearrange("p (r t) -> p r t", r=NCORES), src)
                nc.sync.dma_start(wo_sb[:],
                                  wo_d.rearrange("(ko p) n -> p ko n", p=128))

                with tc.tile_pool(name="kvps", bufs=3, space="PSUM") as kvps:
                    for h in range(HPC):
                        for n4 in range(4):
                            pk = kvps.tile([128, 512], F32, tag="kps")
                            for cc in range(KL // 128):
                                nc.tensor.matmul(pk[:],
                                                 wkvb_sb[:, cc, h * 128:(h + 1) * 128],
                                                 kvaT_sb[cc][:, bass.ts(n4, 512)],
                                                 start=(cc == 0),
                                                 stop=(cc == KL // 128 - 1))
                            nc.scalar.copy(kT[h][:, bass.ts(n4, 512)], pk[:])

                    for tt16 in range(QTILES):
                        for hp in range(2):
                            pv = kvps.tile([128, 256], F32, tag="vps")
                            for cc in range(KL // 128):
                                nc.tensor.matmul(
                                    pv[:],
                                    kvaT_sb[cc][:, tt16 * 128:(tt16 + 1) * 128],
                                    wkvb_sb[:, cc,
                                            HPC * 128 + hp * 256:
                                            HPC * 128 + (hp + 1) * 256],
                                    start=(cc == 0), stop=(cc == KL // 128 - 1))
                            nc.scalar.copy(v_sb[:, tt16, hp * 256:(hp + 1) * 256],
                                           pv[:])

                # q = wqb.T @ q_aT (+ rope on pe rows)
                wqb_sb = proj.tile([128, QL // 128, HPC * 192], F16, tag="wqb")
                nc.sync.dma_start(wqb_sb[:],
                                  wqb_d.rearrange("(ko p) c -> p ko c", p=128))
                qTpeE = proj.tile([128, T], F16, tag="qTpeE")
                qTpeO = proj.tile([128, T], F16, tag="qTpeO")

                with tc.tile_pool(name="qbps", bufs=1, space="PSUM") as qbps:
                    for ts4 in range(4):
                        tsl = bass.ts(ts4, 512)
                        psm = [qbps.tile([128, 512], F32, tag=f"qbps{m}",
                                         name=f"qbps{m}") for m in range(6)]
                        for cc in range(QL // 128):
                            rhs_q = projw.tile([128, 512], F16, tag="rhs_q")
                            src = bass.AP(
                                tensor=ag1_out.tensor,
                                offset=ag1_out.offset
                                + (2 * ts4 * QL + cc * 128) * TPC,
                                ap=[[TPC, 128], [QL * TPC, 2], [1, TPC]])
                            nc.sync.dma_start(
                                rhs_q[:].rearrange("p (r t) -> p r t", r=2), src)
                            for m in range(6):
                                nc.tensor.matmul(psm[m][:],
                                                 wqb_sb[:, cc, m * 128:(m + 1) * 128],
                                                 rhs_q[:], start=(cc == 0),
                                                 stop=(cc == QL // 128 - 1))
                        for m in range(4):
                            nc.scalar.copy(qTn[m][:, tsl], psm[m][:])
                        # rope on packed E/O psums
                        cf, sf = cosF4_sb[:, tsl], sinF4_sb[:, tsl]
                        t1 = projs.tile([128, 512], F32, tag="qrt1")
                        t2 = projs.tile([128, 512], F32, tag="qrt2")
                        nc.vector.tensor_mul(t1[:], psm[4][:], cf)
                        nc.vector.tensor_mul(t2[:], psm[5][:], sf)
                        nc.vector.tensor_sub(qTpeE[:, tsl], t1[:], t2[:])
                        t3 = projs.tile([128, 512], F32, tag="qrt3")
                        t4 = projs.tile([128, 512], F32, tag="qrt4")
                        nc.vector.tensor_mul(t3[:], psm[5][:], cf)
                        nc.vector.tensor_mul(t4[:], psm[4][:], sf)
                        nc.vector.tensor_add(qTpeO[:, tsl], t3[:], t4[:])
                        # repack packed E/O rows -> per-head [E32|O32] contiguous
                        for h in range(HPC):
                            nc.gpsimd.dma_start(qTpe[h][0:32, tsl],
                                                qTpeE[h * 32:(h + 1) * 32, tsl])
                            nc.gpsimd.dma_start(qTpe[h][32:64, tsl],
                                                qTpeO[h * 32:(h + 1) * 32, tsl])


            # ---------------- Stage 2c: causal attention, S^T formulation
            with (
                tc.tile_pool(name="atw", bufs=2) as atw,
                tc.tile_pool(name="atp", bufs=1) as atp,
                tc.tile_pool(name="atps", bufs=3, space="PSUM") as atps,
                tc.tile_pool(name="atpsA", bufs=1, space="PSUM") as atpsA,
            ):
                for h in range(HPC):
                    rsum_row = atp.tile([1, T], F32, tag="rsum_row")
                    for g in range(4):
                        gsl = bass.ts(g, 512)
                        PT_g = atw.tile([128, QTILES, 512], F16, tag="PTg")
                        rs_ps = atps.tile([1, 512], F32, tag="rsps",
                                           bufs=1)
                        nsc = 4 * g + 4
                        for sc in range(nsc):
                            kk = sc - 4 * g
                            v0 = 128 * kk if kk >= 0 else 0
                            pS = atps.tile([128, 512], F32, tag="Sps")
                            q0 = g * 512 + v0
                            nc.tensor.matmul(pS[:, v0:512],
                                             kT[h][:, sc * 128:(sc + 1) * 128],
                                             qTn[h][:, q0:(g + 1) * 512],
                                             start=True, stop=False)
                            nc.tensor.matmul(pS[:, v0:512],
                                             kpeT[:, sc * 128:(sc + 1) * 128],
                                             qTpe[h][:, q0:(g + 1) * 512],
                                             start=False, stop=True)
                            if kk >= 0:
                                nc.vector.tensor_add(pS[:, v0:v0 + 128],
                                                     pS[:, v0:v0 + 128], triT_sb[:])
                            nc.scalar.activation(PT_g[:, sc, v0:512],
                                                 pS[:, v0:512], AF.Exp,
                                                 bias=ebias_sb[:], scale=SM_SCALE)
                            nc.tensor.matmul(rs_ps[:, v0:512], ones16[:],
                                             PT_g[:, sc, v0:512],
                                             start=(sc == 0), stop=(sc == nsc - 1))
                        nc.vector.tensor_copy(rsum_row[:, gsl], rs_ps[:])
                        # PV for the 4 q-tiles of this group
                        pA = [atpsA.tile([128, 128], F32, tag=f"Aps{qq}",
                                         name=f"Aps{qq}") for qq in range(4)]
                        for sc in range(nsc):
                            for qq in range(4):
                                qt = 4 * g + qq
                                if qt < sc:
                                    continue
                                nc.tensor.matmul(pA[qq][:],
                                                 v_sb[:, sc, h * 128:(h + 1) * 128],
                                                 PT_g[:, sc, qq * 128:(qq + 1) * 128],
                                                 start=(sc == 0), stop=(sc == qt))
                        for qq in range(4):
                            qt = 4 * g + qq
                            nc.vector.tensor_copy(
                                attnT[h][:, qt * 128:(qt + 1) * 128], pA[qq][:])

                    # normalize: attnT[h] *= (1/rowsum) broadcast along partitions
                    rec32 = atp.tile([1, T], F32, tag="rec32")
                    nc.vector.reciprocal(rec32[:], rsum_row[:])
                    rec16 = atp.tile([1, T], F16, tag="rec16")
                    nc.scalar.copy(rec16[:], rec32[:])
                    rscr = dram2.tile([1, T], F16, tag="rscr")
                    nc.gpsimd.dma_start(rscr[:], rec16[:])
                    rrec = atw.tile([128, T], F16, tag="rrec")
                    bcast = bass.AP(tensor=rscr.tensor, offset=rscr.offset,
                                    ap=[[0, 128], [1, T]])
                    nc.gpsimd.dma_start(rrec[:], bcast)
                    nc.vector.tensor_mul(attnT[h][:], attnT[h][:], rrec[:])

            # ---------------- Stage 2d: partial o_proj for local heads
            with (
                tc.tile_pool(name="os", bufs=4) as osb_pool,
                tc.tile_pool(name="ops", bufs=4, space="PSUM") as ops,
            ):
                for n10 in range(10):
                    for m in range(QTILES):
                        po = ops.tile([128, 512], F32, tag="ops")
                        for cc in range(HPC):
                            nc.tensor.matmul(
                                po[:], attnT[cc][:, m * 128:(m + 1) * 128],
                                wo_sb[:, cc, n10 * 512:(n10 + 1) * 512],
                                start=(cc == 0), stop=(cc == HPC - 1))
                        osb = osb_pool.tile([128, 512], F16, tag="osb")
                        if m % 2 == 0:
                            nc.scalar.copy(osb[:], po[:])
                        else:
                            nc.vector.tensor_copy(osb[:], po[:])
                        nc.sync.dma_start(
                            out_d[m * 128:(m + 1) * 128, bass.ts(n10, 512)], osb[:])

    nc.compile()
    _PROGRAM_CACHE["nc"] = nc
    return nc


def _host_prep(inputs):
    pos = np.asarray(inputs["positions"]).astype(np.float32)
    inv_freq = 1.0 / (THETA ** (np.arange(0, DR, 2, dtype=np.float32) / DR))
    freqs = pos[:, None] * inv_freq[None, :]
    cos, sin = np.cos(freqs), np.sin(freqs)

    eo = np.concatenate([np.arange(0, DR, 2), np.arange(1, DR, 2)])
    w_qkv_a = np.asarray(inputs["w_qkv_a"], np.float32).copy()
    w_qkv_a[:, QL + KL:] = w_qkv_a[:, QL + KL:][:, eo]
    w_q_b = np.asarray(inputs["w_q_b"], np.float32) * np.asarray(
        inputs["q_a_ln_w"], np.float32)[:, None]
    w_kv_b = np.asarray(inputs["w_kv_b"], np.float32) * np.asarray(
        inputs["kv_a_ln_w"], np.float32)[:, None]
    w_o = np.asarray(inputs["w_o"], np.float32)
    hidT = np.ascontiguousarray(np.asarray(inputs["hidden_states"], np.float32).T)

    wa16 = w_qkv_a.astype(np.float16)
    cosF4 = np.ascontiguousarray(np.tile(cos.T, (4, 1))).astype(np.float32)
    sinF4 = np.ascontiguousarray(np.tile(sin.T, (4, 1))).astype(np.float32)
    triT = np.tril(np.full((128, 128), NEG, np.float32), -1)

    in_maps = []
    for c in range(NCORES):
        hs = [HPC * c + i for i in range(HPC)]
        nope_cols = np.concatenate(
            [w_q_b[:, h * 192:h * 192 + DN] for h in hs], axis=1)
        peE = np.concatenate(
            [w_q_b[:, h * 192 + DN:(h + 1) * 192][:, eo[:32]] for h in hs], axis=1)
        peO = np.concatenate(
            [w_q_b[:, h * 192 + DN:(h + 1) * 192][:, eo[32:]] for h in hs], axis=1)
        wqb_c = np.concatenate([nope_cols, peE, peO], axis=1)
        kcols = np.concatenate(
            [w_kv_b[:, h * 256:h * 256 + DN] for h in hs], axis=1)
        vcols = np.concatenate(
            [w_kv_b[:, h * 256 + DN:(h + 1) * 256] for h in hs], axis=1)
        wkvb_c = np.concatenate([kcols, vcols], axis=1)
        wo_c = np.concatenate([w_o[h * DV:(h + 1) * DV, :] for h in hs], axis=0)
        sl = slice(c * TPC, (c + 1) * TPC)
        in_maps.append({
            "hT": np.ascontiguousarray(hidT[:, sl]).astype(np.float16),
            "wa": wa16,
            "wqb": np.ascontiguousarray(wqb_c).astype(np.float16),
            "wkvb": np.ascontiguousarray(wkvb_c).astype(np.float16),
            "wo": np.ascontiguousarray(wo_c).astype(np.float16),
            "ctok": np.ascontiguousarray(cos[sl]).astype(np.float16),
            "stok": np.ascontiguousarray(sin[sl]).astype(np.float16),
            "cosF4": cosF4,
            "sinF4": sinF4,
            "triT": triT,
        })
    return in_maps


def kernel(**inputs) -> np.ndarray:
    nc = build_program()
    in_maps = _host_prep(inputs)
    res = run_bass_kernel_spmd(nc, in_maps, core_ids=list(range(NCORES)))
    out = np.zeros((T, HID), np.float32)
    for r in res.results:
        out += r["out"].astype(np.float32)
    return out


if __name__ == "__main__":
    build_program()
    print("program built ok")

